# revision 9
# baseline (speedup 1.0000x reference)
"""ComplexPolarAttention Trainium2 kernel (8-core SPMD, row-sharded).

Math (matching the reference):
  c = mag*cos(phase); s = mag*sin(phase)
  scores = c@c.T + s@s.T + bias     (bias: sparse edge scatter, last-dup-wins)
  attn = softmax(scores, axis=1)
  out = (attn@mag, attn@phase)

Host precomputes everything elementwise-cheap: the trig features packed
transposed as xt [128 feat, 8192 nodes] f16, the PV value matrix
mp [128, 64*132] bf16 ([mag|phase|ones] per key chunk), and the scalar
edge scores es = edge_attr@W.sum(0)+b.sum() bucketed for the device
scatter. The device then runs a pure attention pipeline per core
(1024 query rows), qb-major (4 query blocks x 16 groups of 4 key chunks):

  PE:     S^T group tile [128 dst, 4kc x 256 q] = xt_kc.T @ xtq   (f16)
  GpSimd: dense bias tile via local_scatter of es values
  DVE:    tmp = S^T + bias (f32, SBUF), quad-batched [128, 4096]
  ACT:    P = exp(tmp) -> bf16, FD=4096 per instruction
  PE:     pv[128 q, 129] += P_chunk.T @ [mag|phase|ones]  (col 128 =
          softmax denominator); epilogue divides and DMAs out.

QK matmuls for quad q+1 are emitted before PV matmuls of quad q so the
tensor-engine queue never stalls on the exp.
"""
import os
import sys

sys.path.insert(0, "/opt/trn_rl_repo")

# The NTFF profile hook module is missing from this image's antenv package;
# bass_utils imports it unconditionally when tracing. Create it if absent so
# BASS_TRACE=1 works (degrades silently if dirs are read-only).
_HOOK_SRC = '''_hook = None

def set_axon_ntff_profile_hook(hook):
    global _hook
    _hook = hook

def get_axon_ntff_profile_hook():
    return _hook
'''
for _d in ("/opt/trn_rl_repo/antenv", "/root/.axon_site/_ro/trn_rl_repo/antenv"):
    try:
        _p = os.path.join(_d, "axon_hooks.py")
        if os.path.isdir(_d) and not os.path.exists(_p):
            with open(_p, "w") as _f:
                _f.write(_HOOK_SRC)
    except OSError:
        pass

import numpy as np
import ml_dtypes

import concourse.bass as bass
import concourse.mybir as mybir
import concourse.tile as tile
from concourse import bacc
from concourse.bass_utils import run_bass_kernel_spmd
from concourse.masks import make_identity

N, D, E, EDGE_DIM = 8192, 64, 262144, 4
CORES = 8
NQ = N // CORES          # 1024 query rows per core
QB_W = 256               # query block width
N_QB = NQ // QB_W        # 4 query blocks per core
KC = 128                 # key chunk (dst) width
N_KC = N // KC           # 64 key chunks
KCG = 4                  # key chunks per scatter/exp group
N_G = N_KC // KCG        # 16 groups per qb
GW = KCG * QB_W          # 1024 = group tile width
QUAD = 4                 # groups per exp instruction
N_Q4 = N_G // QUAD       # 4 quads per qb
MPW = 132                # padded [mag|phase|ones] chunk stride

f32 = mybir.dt.float32
f16 = mybir.dt.float16
bf16 = mybir.dt.bfloat16
i16 = mybir.dt.int16
AF = mybir.ActivationFunctionType
ALU = mybir.AluOpType

_cache = {}
LAST_RESULTS = None


def _build(slots):
    tot = N_QB * N_G * slots
    nc = bacc.Bacc("TRN2", target_bir_lowering=False, debug=False,
                   num_devices=CORES)
    xt_d = nc.dram_tensor("xt", (128, N), f16, kind="ExternalInput")
    xtq_d = nc.dram_tensor("xtq", (128, NQ), f16, kind="ExternalInput")
    mp_d = nc.dram_tensor("mp", (128, N_KC * MPW), bf16, kind="ExternalInput")
    eidx_d = nc.dram_tensor("eidx", (128, tot), i16, kind="ExternalInput")
    esb_d = nc.dram_tensor("esb", (128, tot), f16, kind="ExternalInput")
    out_d = nc.dram_tensor("out", (NQ, 128), f32, kind="ExternalOutput")

    with tile.TileContext(nc) as tc, \
         tc.tile_pool(name="persist", bufs=1) as pers:
        xt = pers.tile([128, N], f16, tag="xt")
        xtq = pers.tile([128, NQ], f16, tag="xtq")
        mp = pers.tile([128, N_KC * MPW], bf16, tag="mp")
        esb = pers.tile([128, tot], f16, tag="esb")
        eidx_sb = pers.tile([128, tot], i16, tag="eidx_sb")
        ident = pers.tile([128, 128], f16, tag="ident")
        make_identity(nc, ident[:])

        # Inputs the first groups need come first; DMAs are spread across
        # the engine DGE queues (engines are idle at startup) so transfers
        # run in parallel instead of serializing on the sync queue.
        nc.sync.dma_start(out=xtq[:], in_=xtq_d[:])
        nc.gpsimd.dma_start(out=esb[:], in_=esb_d[:])
        nc.gpsimd.dma_start(out=eidx_sb[:], in_=eidx_d[:])
        NCH = 8
        for h in range(NCH):
            a, b = h * (N // NCH), (h + 1) * (N // NCH)
            nc.sync.dma_start(out=xt[:, a:b], in_=xt_d[:, a:b])
            am, bm = h * (N_KC * MPW // NCH), (h + 1) * (N_KC * MPW // NCH)
            nc.scalar.dma_start(out=mp[:, am:bm], in_=mp_d[:, am:bm])

        with tc.tile_pool(name="qk_ps", bufs=3, space="PSUM") as qkp, \
             tc.tile_pool(name="pv_ps", bufs=1, space="PSUM") as pvp, \
             tc.tile_pool(name="psb", bufs=2) as psbp, \
             tc.tile_pool(name="tmp", bufs=2) as tmpp, \
             tc.tile_pool(name="bias", bufs=8) as biasp, \
             tc.tile_pool(name="epi", bufs=2) as epip:

            def emit_qk(qb, q):
                """QK matmuls for one quad; returns the 4 psum tiles.

                The last quad of each qb takes its bias on the tensor engine
                (identity-matmul accumulate into PSUM) to offload the vector
                engine, which is the steady-state bottleneck."""
                pe_bias = (q == N_Q4 - 1)
                tiles = []
                for gl in range(QUAD):
                    g = q * QUAD + gl
                    qk = qkp.tile([128, GW], f32, tag="qk")
                    bias_t = None
                    if pe_bias:
                        bias_t = biasp.tile([128, GW], f16, tag="bias_t")
                        off = (qb * N_G + g) * slots
                        nc.gpsimd.local_scatter(
                            bias_t[:], esb[:, off:off + slots],
                            eidx_sb[:, off:off + slots],
                            channels=128, num_elems=GW, num_idxs=slots)
                    for j in range(KCG):
                        kc = g * KCG + j
                        sl = slice(j * QB_W, (j + 1) * QB_W)
                        nc.tensor.matmul(
                            out=qk[:, sl],
                            lhsT=xt[:, kc * 128:(kc + 1) * 128],
                            rhs=xtq[:, qb * QB_W:(qb + 1) * QB_W],
                            start=True, stop=not pe_bias)
                        if pe_bias:
                            # accumulate the scattered bias right after its
                            # slice's QK matmul: a start=True on another
                            # slice of the same tile in between breaks the
                            # PSUM accumulation group (probe-verified)
                            nc.tensor.matmul(
                                out=qk[:, sl], lhsT=ident[:],
                                rhs=bias_t[:, sl], start=False, stop=True)
                    tiles.append(qk)
                return tiles

            pend = None
            for qb in range(N_QB):
                p_sb = psbp.tile([128, N_G * GW], bf16, tag="p_sb")
                pv0 = pvp.tile([128, 129], f32, tag="pv0")
                pv1 = pvp.tile([128, 129], f32, tag="pv1")
                for q in range(N_Q4):
                    qk_tiles = pend if pend is not None else emit_qk(qb, q)
                    pend = None
                    if q == N_Q4 - 1:
                        # bias already accumulated in PSUM by the PE; exp
                        # straight from PSUM, one group at a time
                        for gl in range(QUAD):
                            c0 = (q * QUAD + gl) * GW
                            nc.scalar.activation(
                                out=p_sb[:, c0:c0 + GW],
                                in_=qk_tiles[gl][:], func=AF.Exp)
                    else:
                        tmp = tmpp.tile([128, QUAD * GW], f32, tag="tmp")
                        for gl in range(QUAD):
                            g = q * QUAD + gl
                            bias_t = biasp.tile([128, GW], f16, tag="bias_t")
                            off = (qb * N_G + g) * slots
                            nc.gpsimd.local_scatter(
                                bias_t[:], esb[:, off:off + slots],
                                eidx_sb[:, off:off + slots],
                                channels=128, num_elems=GW, num_idxs=slots)
                            nc.vector.tensor_tensor(
                                out=tmp[:, gl * GW:(gl + 1) * GW],
                                in0=qk_tiles[gl][:], in1=bias_t[:], op=ALU.add)
                        nc.scalar.activation(
                            out=p_sb[:, q * QUAD * GW:(q + 1) * QUAD * GW],
                            in_=tmp[:], func=AF.Exp)
                    # queue next quad's QK ahead of this quad's PV so the
                    # tensor engine never waits on the exp
                    if q + 1 < N_Q4:
                        pend = emit_qk(qb, q + 1)
                    elif qb + 1 < N_QB:
                        pend = emit_qk(qb + 1, 0)
                    for gl in range(QUAD):
                        g = q * QUAD + gl
                        for j in range(KCG):
                            kc = g * KCG + j
                            col = g * GW + j * QB_W
                            for qs, pv in ((0, pv0), (1, pv1)):
                                nc.tensor.matmul(
                                    out=pv[:],
                                    lhsT=p_sb[:, col + qs * 128:
                                              col + (qs + 1) * 128],
                                    rhs=mp[:, kc * MPW:kc * MPW + 2 * D + 1],
                                    start=(kc == 0), stop=(kc == N_KC - 1))
                for qs, pv in ((0, pv0), (1, pv1)):
                    rec = epip.tile([128, 1], f32, tag=f"rec{qs}")
                    nc.vector.reciprocal(out=rec[:], in_=pv[:, 128:129])
                    o_t = epip.tile([128, 128], f32, tag=f"o_t{qs}")
                    nc.vector.tensor_scalar(o_t[:], pv[:, 0:128], rec[:], None,
                                            ALU.mult)
                    r0 = qb * QB_W + qs * 128
                    nc.sync.dma_start(out=out_d[r0:r0 + 128, :], in_=o_t[:])

    nc.compile()
    return nc


def _prep_edges(edge_index, es):
    """Dedup (last wins, matching CPU XLA scatter-set) and bucket edges.

    Layout per core: cell = (qb, g, p) with qb = src query block (256 rows),
    g = dst group (4 key chunks = 512 dst), p = dst % 128; the scattered
    column inside the [128, 1024] group tile is ((dst%512)//128)*256 +
    src%256. Values are the host-computed edge scores (f16)."""
    src = np.asarray(edge_index[0], dtype=np.int64)
    dst = np.asarray(edge_index[1], dtype=np.int64)
    keys = src * N + dst
    order = np.argsort(keys, kind="stable")
    ks = keys[order]
    run_last = np.flatnonzero(np.r_[ks[1:] != ks[:-1], True])
    kept = order[run_last]  # stable sort => last occurrence per duplicate key
    s, d = src[kept], dst[kept]
    vals = es[kept]

    core = s // NQ
    qb = (s % NQ) // QB_W
    g = d // (KCG * KC)
    p = d % 128
    col = ((d % (KCG * KC)) // KC) * QB_W + (s % QB_W)

    cell = ((core * N_QB + qb) * N_G + g) * 128 + p
    o2 = np.argsort(cell, kind="stable")
    cell_s = cell[o2]
    first = np.r_[True, cell_s[1:] != cell_s[:-1]]
    run_id = np.cumsum(first) - 1
    run_start = np.flatnonzero(first)
    slot = np.arange(len(cell_s)) - run_start[run_id]
    slots = int(max(int(slot.max()) + 1 if len(slot) else 1, 4))
    slots = (slots + 1) // 2 * 2  # even

    tot = N_QB * N_G * slots
    eidx = np.full((CORES, 128, tot), -1, dtype=np.int16)
    esb = np.zeros((CORES, 128, tot), dtype=np.float16)
    cs, qbs, gs, ps, cols = core[o2], qb[o2], g[o2], p[o2], col[o2]
    off = (qbs * N_G + gs) * slots + slot
    eidx[cs, ps, off] = cols.astype(np.int16)
    esb[cs, ps, off] = vals[o2].astype(np.float16)
    return eidx, esb, slots


def kernel(mag, phase, edge_index, edge_attr, W, b):
    global LAST_RESULTS
    mag = np.asarray(mag, dtype=np.float32)
    phase = np.asarray(phase, dtype=np.float32)
    W = np.asarray(W, dtype=np.float32)
    bv = np.asarray(b, dtype=np.float32)

    # trig features, packed transposed: xt[[cos|sin] x d, node]
    c = (mag * np.cos(phase)).astype(np.float16)
    s = (mag * np.sin(phase)).astype(np.float16)
    xt = np.ascontiguousarray(np.concatenate([c.T, s.T], axis=0))  # [128, N]

    # PV value matrix per key chunk: [mag | phase | 1 | pad] stride 132
    mp = np.zeros((128, N_KC, MPW), dtype=np.float32)
    mp[:, :, 0:D] = mag.reshape(N_KC, 128, D).transpose(1, 0, 2)
    mp[:, :, D:2 * D] = phase.reshape(N_KC, 128, D).transpose(1, 0, 2)
    mp[:, :, 2 * D] = 1.0
    mp = mp.reshape(128, N_KC * MPW).astype(ml_dtypes.bfloat16)

    # scalar edge scores: sum_h (edge_attr @ W.T + b)[:, h]
    es = (np.asarray(edge_attr, dtype=np.float64) @
          W.astype(np.float64).sum(axis=0) + bv.astype(np.float64).sum())
    eidx, esb, slots = _prep_edges(edge_index, es)

    if slots not in _cache:
        _cache[slots] = _build(slots)
    nc = _cache[slots]

    in_maps = []
    for cid in range(CORES):
        in_maps.append({
            "xt": xt,
            "xtq": np.ascontiguousarray(xt[:, cid * NQ:(cid + 1) * NQ]),
            "mp": mp,
            "eidx": np.ascontiguousarray(eidx[cid]),
            "esb": np.ascontiguousarray(esb[cid]),
        })
    res = run_bass_kernel_spmd(nc, in_maps, core_ids=list(range(CORES)))
    LAST_RESULTS = res

    new_mag = np.empty((N, D), dtype=np.float32)
    new_phase = np.empty((N, D), dtype=np.float32)
    for cid in range(CORES):
        o = res.results[cid]["out"]
        new_mag[cid * NQ:(cid + 1) * NQ] = o[:, 0:D]
        new_phase[cid * NQ:(cid + 1) * NQ] = o[:, D:2 * D]
    return new_mag, new_phase
